# revision 1
# baseline (speedup 1.0000x reference)
"""DMPNN encoder on 8 TRN2 cores via Bass/Tile.

Design (per core, node-sharded npc nodes/core):
  sweep1 (src-token order): recompute input_msg via Wi matmuls from
    transpose-gathered atom features; messages_1 = relu(imsg);
    one-hot S matmuls accumulate A^T_2; Wh tail -> B2 slice; AllGather -> Bfull2.
  sweep2 (src order): gather Bfull2[dst] rows; messages_2 = relu(imsg + g);
    -> A^T_3 -> Wh -> B3 (local only, no collective).
  sweep3 (dst order): gather B3[dst_local]; messages_3 = relu(imsg + g);
    S_dst matmuls -> in_agg^T; readout: Wo matmuls (+mask/bias baked into
    atomT_read), masked node sums -> mol partials; AllReduce; Wout -> out.

Biases: bi and bh(+has_nb) baked into Wi via constant-1 / has_nb indicator
features; B tables are pure A @ Wh.T so pad rows are zero (zero-row trick for
has_nb=0 edges and pad tokens). bo baked via mask row of atomT_read.
Self-loop correction is skipped (error ~5e-6, verified in proto.py).
"""
import numpy as np
import ml_dtypes

BF16 = ml_dtypes.bfloat16

NODE_F = 117
EDGE_F = 10
H = 300
DEPTH = 3


# ---------------------------------------------------------------- host side

def _grow(v, npc, NPC):
    return (v // npc) * NPC + (v % npc)


def _pack_idx(idx):
    """[TOK] int -> [128, TOK/16] int16 in dma_gather wrap layout."""
    idx = np.asarray(idx, np.int64)
    assert len(idx) % 16 == 0
    a = idx.reshape(-1, 16).T.astype(np.int16)
    assert (idx < 32768).all() and (idx >= 0).all()
    return np.tile(a, (8, 1))


def preprocess(atom, ef, src, dst, Wi, bi, Wh, bh, Wo, bo, Wout, bout, C=8, gblk=20):
    N, E = atom.shape[0], src.shape[0]
    assert N % C == 0
    npc = N // C
    TPB = npc // 128 + 1          # always >= 1 pad row per core
    NPC = TPB * 128
    GROWS = C * NPC
    HALFW = (C // 2) * NPC
    assert HALFW <= 32768
    ZR = npc                      # local zero-row index (first pad row)

    deg_src = np.bincount(src, minlength=N)
    self_loop = src == dst
    has_nb = (deg_src[dst] - self_loop.astype(np.int64)) > 0
    deg_in = np.bincount(dst, minlength=N)

    meta = dict(C=C, N=N, E=E, npc=npc, TPB=TPB, NPC=NPC, GROWS=GROWS,
                HALFW=HALFW, ZR=ZR, orders={})
    percore = [dict() for _ in range(C)]

    # ---- shared tables
    atom_g = np.zeros((GROWS, 128), BF16)
    gr = _grow(np.arange(N), npc, NPC)
    atom_g[gr, :NODE_F] = atom.astype(BF16)

    for c in range(C):
        lo = c * npc
        al = np.zeros((NPC, 128), BF16)
        al[:npc, :NODE_F] = atom[lo:lo + npc].astype(BF16)
        percore[c]["atom_l"] = al
        # masked transposed readout table with mask row 127 (bakes bo + deg_in mask)
        atr = np.zeros((128, NPC), BF16)
        msk = (deg_in[lo:lo + npc] > 0)
        atr[:NODE_F, :npc] = (atom[lo:lo + npc].T * msk[None, :]).astype(BF16)
        atr[127, :npc] = msk.astype(BF16)
        percore[c]["atomT_read"] = atr

    # ---- weights (shared, replicated)
    shared = {"atom_g": atom_g}
    wi_atom = np.zeros((128, 384), BF16)
    wi_atom[:NODE_F, :H] = Wi[:, :NODE_F].T.astype(BF16)
    wi_ef = np.zeros((16, 384), BF16)
    wi_ef[:EDGE_F, :H] = Wi[:, NODE_F:].T.astype(BF16)
    wi_ef[10, :H] = bi.astype(BF16)
    wi_ef[11, :H] = bh.astype(BF16)
    shared["wi_atom"] = wi_atom
    shared["wi_ef"] = wi_ef
    wht = np.zeros((320, 384), np.float32)
    wht[:H, :H] = Wh.T.astype(np.float32)
    shared["wht0"] = wht[0:128]
    shared["wht1"] = wht[128:256]
    shared["wht2"] = wht[256:320]
    wo = np.zeros((448, 384), BF16)   # K rows: 0..127 atom(+mask@127), 128.. in_agg
    wo[:NODE_F, :H] = Wo[:, :NODE_F].T.astype(BF16)
    wo[127, :H] = bo.astype(BF16)
    wo[128:128 + H, :H] = Wo[:, NODE_F:].T.astype(BF16)
    shared["wo_ka"] = wo[0:128]
    shared["wo_k2"] = wo[128:256]
    shared["wo_k3"] = wo[256:384]
    shared["wo_k4"] = wo[384:448]
    wout = np.zeros((384, 320), np.float32)
    wout[:H, :H] = (Wout.T / N).astype(np.float32)
    shared["woutt0"] = wout[0:128]
    shared["woutt1"] = wout[128:256]
    shared["woutt2"] = np.concatenate([wout[256:384], np.zeros((0, 320), np.float32)])
    shared["bout_row"] = np.pad(bout.astype(np.float32), (0, 20))[None, :]
    shared["one_t"] = np.ones((1, 1), np.float32)

    # ---- per-order token layouts
    for order in ("src", "dst"):
        key = src if order == "src" else dst
        owner = key // npc
        loc = key - owner * npc
        tile_of = loc // 128
        halves = 2 if order == "src" else 1
        if order == "src":
            gd = _grow(dst, npc, NPC)
            half_of = gd // HALFW
        else:
            half_of = np.zeros(E, np.int64)

        # vectorized per-(core, h, t) bucketing
        gid = (owner * halves + half_of) * TPB + tile_of
        counts = np.bincount(gid, minlength=C * halves * TPB).reshape(C, halves, TPB)
        n_chunks = -(-counts.max(axis=0) // 128)  # [halves, TPB]
        blk0 = np.zeros((halves, TPB), np.int64)
        acc = 0
        for h in range(halves):
            for t in range(TPB):
                blk0[h, t] = acc
                acc += n_chunks[h, t]
        TOTBLK = int(acc)
        TOK = TOTBLK * 128

        # gather-call groups: contiguous tiles within a half, ~gblk chunks each
        groups = []
        for h in range(halves):
            t = 0
            while t < TPB:
                t0, nb = t, 0
                while t < TPB and (nb == 0 or nb + n_chunks[h, t] <= gblk):
                    nb += n_chunks[h, t]
                    t += 1
                if nb:
                    groups.append(dict(h=h, t0=t0, t1=t, b0=int(blk0[h, t0]),
                                       b1=int(blk0[h, t - 1] + n_chunks[h, t - 1])))
        om = dict(halves=halves, n_chunks=n_chunks, blk0=blk0, TOTBLK=TOTBLK,
                  TOK=TOK, groups=groups)
        meta["orders"][order] = om

        # vectorized token assignment: stable-sort edges by gid, position within
        # group + per-(h,t) chunk base gives each edge its token slot
        ordr = np.argsort(gid, kind="stable")
        sorted_gid = gid[ordr]
        grp_starts = np.searchsorted(sorted_gid, np.arange(C * halves * TPB))
        within = np.arange(E) - grp_starts[sorted_gid]
        base_tok = np.broadcast_to((blk0 * 128)[None], (C, halves, TPB)).reshape(-1)
        tok_sorted = base_tok[sorted_gid] + within
        tok = np.empty(E, np.int64)
        tok[ordr] = tok_sorted

        if order == "src":
            vA = gd - half_of * HALFW
        else:
            vA = loc.copy()
        vB = np.where(has_nb, vA, ZR)

        for c in range(C):
            sel = owner == c
            tk = tok[sel]
            idxA = np.full(TOK, ZR, np.int64)
            idxB = np.full(TOK, ZR, np.int64)
            idxA[tk] = vA[sel]
            idxB[tk] = vB[sel]
            efT = np.zeros((16, TOK), BF16)
            efT[:EDGE_F, tk] = ef[sel].T.astype(BF16)
            efT[10, tk] = 1.0
            efT[11, tk] = has_nb[sel].astype(BF16)
            S = np.zeros((128, TOTBLK, 128), BF16)
            S[tk % 128, tk // 128, (loc[sel] - tile_of[sel] * 128)] = 1.0
            percore[c][f"idxA_{order}"] = _pack_idx(idxA)
            percore[c][f"idxB_{order}"] = _pack_idx(idxB)
            percore[c][f"efT_{order}"] = efT
            percore[c][f"S_{order}"] = S

    in_maps = []
    for c in range(C):
        m = dict(shared)
        m.update(percore[c])
        in_maps.append(m)
    return meta, in_maps


# ---------------------------------------------------------------- device side

def build_nc(meta, debug=False, dump=False):
    import concourse.bass as bass
    import concourse.tile as tile
    from concourse import bacc, mybir
    from concourse.library_config import mlp

    C, NPC, TPB = meta["C"], meta["NPC"], meta["TPB"]
    GROWS, HALFW, npc = meta["GROWS"], meta["HALFW"], meta["npc"]
    f32, bf16, i16 = mybir.dt.float32, mybir.dt.bfloat16, mybir.dt.int16

    nc = bacc.Bacc("TRN2", target_bir_lowering=False, debug=debug, num_devices=C)

    def din(name, shape, dt):
        return nc.dram_tensor(name, shape, dt, kind="ExternalInput")

    oms = meta["orders"]
    atom_g = din("atom_g", [GROWS, 128], bf16)
    atom_l = din("atom_l", [NPC, 128], bf16)
    atomT_read = din("atomT_read", [128, NPC], bf16)
    ins = {}
    for o in ("src", "dst"):
        om = oms[o]
        ins[f"idxA_{o}"] = din(f"idxA_{o}", [128, om["TOK"] // 16], i16)
        ins[f"idxB_{o}"] = din(f"idxB_{o}", [128, om["TOK"] // 16], i16)
        ins[f"efT_{o}"] = din(f"efT_{o}", [16, om["TOK"]], bf16)
        ins[f"S_{o}"] = din(f"S_{o}", [128, om["TOTBLK"], 128], bf16)
    wi_atom = din("wi_atom", [128, 384], bf16)
    wi_ef = din("wi_ef", [16, 384], bf16)
    wht = [din(f"wht{i}", [128 if i < 2 else 64, 384], f32) for i in range(3)]
    wo_ka = din("wo_ka", [128, 384], bf16)
    wo_k2 = din("wo_k2", [128, 384], bf16)
    wo_k3 = din("wo_k3", [128, 384], bf16)
    wo_k4 = din("wo_k4", [64, 384], bf16)
    woutt = [din(f"woutt{i}", [128, 320], f32) for i in range(3)]
    bout_row = din("bout_row", [1, 320], f32)
    one_t = din("one_t", [1, 1], f32)
    out_d = nc.dram_tensor("out", [1, 320], f32, kind="ExternalOutput")
    dbg = {}
    if dump:
        for nm in ("at1", "at2", "at3"):
            dbg[nm] = nc.dram_tensor(f"dbg_{nm}", [128, 3 * NPC], f32,
                                     kind="ExternalOutput")
        dbg["b2"] = nc.dram_tensor("dbg_b2", [NPC, 384], bf16, kind="ExternalOutput")
        dbg["bfull"] = nc.dram_tensor("dbg_bfull", [GROWS, 384], bf16,
                                      kind="ExternalOutput")
        dbg["b3"] = nc.dram_tensor("dbg_b3", [NPC, 384], bf16, kind="ExternalOutput")
        dbg["msg0"] = nc.dram_tensor("dbg_msg0", [128, 384], bf16, kind="ExternalOutput")
        dbg["atT0"] = nc.dram_tensor("dbg_atT0", [128, 128], bf16, kind="ExternalOutput")

    with tile.TileContext(nc) as tc:
        nc.gpsimd.load_library(mlp)
        import contextlib
        ctx = contextlib.ExitStack()
        with ctx:
            cpool = ctx.enter_context(tc.tile_pool(name="consts", bufs=1))
            idxpool = ctx.enter_context(tc.tile_pool(name="idx", bufs=1))
            atpool = ctx.enter_context(tc.tile_pool(name="atT", bufs=2))
            efpool = ctx.enter_context(tc.tile_pool(name="efT", bufs=2))
            spool = ctx.enter_context(tc.tile_pool(name="S", bufs=2))
            gpool = ctx.enter_context(tc.tile_pool(name="gB", bufs=2))
            mpool = ctx.enter_context(tc.tile_pool(name="msg", bufs=3))
            accpool = ctx.enter_context(tc.tile_pool(name="ATacc", bufs=1))
            smallpool = ctx.enter_context(tc.tile_pool(name="small", bufs=4))
            ps_big = ctx.enter_context(tc.tile_pool(name="ps_big", bufs=2, space="PSUM"))
            ps_at = ctx.enter_context(tc.tile_pool(name="ps_at", bufs=2, space="PSUM"))
            dram = ctx.enter_context(tc.tile_pool(name="dram", bufs=1, space="DRAM"))

            def cload(t, shape, dt):
                s = cpool.tile(shape, dt, tag=t.name)
                nc.sync.dma_start(s[:], t[:])
                return s

            wi_atom_s = cload(wi_atom, [128, 384], bf16)
            wi_ef_s = cload(wi_ef, [16, 384], bf16)
            wht_s = [cload(w, [128 if i < 2 else 64, 384], f32) for i, w in enumerate(wht)]
            wo_ka_s = cload(wo_ka, [128, 384], bf16)
            wo_k2_s = cload(wo_k2, [128, 384], bf16)
            wo_k3_s = cload(wo_k3, [128, 384], bf16)
            wo_k4_s = cload(wo_k4, [64, 384], bf16)
            woutt_s = [cload(w, [128, 320], f32) for w in woutt]
            bout_s = cload(bout_row, [1, 320], f32)
            one_s = cload(one_t, [1, 1], f32)

            B2 = dram.tile([NPC, 384], bf16)
            Bfull = dram.tile([GROWS, 384], bf16)
            B3 = dram.tile([NPC, 384], bf16)

            def sweep(k):
                order = "src" if k < 3 else "dst"
                om = oms[order]
                idxA = idxpool.tile([128, om["TOK"] // 16], i16, tag="idxA")
                nc.sync.dma_start(idxA[:], ins[f"idxA_{order}"][:])
                idxB = None
                if k > 1:
                    idxB = idxpool.tile([128, om["TOK"] // 16], i16, tag="idxB")
                    nc.sync.dma_start(idxB[:], ins[f"idxB_{order}"][:])
                ATacc = accpool.tile([128, 3 * NPC], f32, tag="ATacc")
                nc.vector.memset(ATacc[:], 0.0)
                gtab = Bfull if k == 2 else B3
                for g in om["groups"]:
                    h, b0, b1 = g["h"], g["b0"], g["b1"]
                    nb = b1 - b0
                    ntok = nb * 128
                    atT = atpool.tile([128, 1, ntok], bf16, tag="atT")
                    asrc = atom_g[h * HALFW:(h + 1) * HALFW, :] if order == "src" \
                        else atom_l[:, :]
                    nc.gpsimd.dma_gather(
                        atT[:], asrc, idxA[:, b0 * 8:b0 * 8 + ntok // 16],
                        ntok, ntok, 128, transpose=True, single_packet=False)
                    efT = efpool.tile([16, ntok], bf16, tag="efT")
                    nc.sync.dma_start(efT[:], ins[f"efT_{order}"][:, b0 * 128:b1 * 128])
                    Ssb = spool.tile([128, nb, 128], bf16, tag="S")
                    nc.sync.dma_start(Ssb[:], ins[f"S_{order}"][:, b0:b1, :])
                    gB = None
                    if k > 1:
                        gB = gpool.tile([128, nb, 384], bf16, tag="gB")
                        gsrc = gtab[h * HALFW:(h + 1) * HALFW, :] if (k == 2) \
                            else gtab[:, :]
                        nc.gpsimd.dma_gather(
                            gB[:], gsrc, idxB[:, b0 * 8:b0 * 8 + ntok // 16],
                            ntok, ntok, 384, single_packet=False)
                    for t in range(g["t0"], g["t1"]):
                        nchk = int(om["n_chunks"][h][t])
                        if nchk == 0:
                            continue
                        at_ps = ps_at.tile([128, 384], f32, tag="at_ps")
                        jb0 = int(om["blk0"][h][t])
                        for jj in range(nchk):
                            j = jb0 + jj
                            jr = j - b0
                            im_ps = ps_big.tile([128, 384], f32, tag="big")
                            nc.tensor.matmul(
                                im_ps[:], atT[:, 0, jr * 128:(jr + 1) * 128],
                                wi_atom_s[:], start=True, stop=False,
                                skip_group_check=True)
                            nc.tensor.matmul(
                                im_ps[:], efT[:, jr * 128:(jr + 1) * 128],
                                wi_ef_s[:], start=False, stop=True,
                                skip_group_check=True)
                            msg = mpool.tile([128, 384], bf16, tag="msg")
                            if k == 1:
                                nc.vector.tensor_scalar(
                                    msg[:], im_ps[:], 0.0, None,
                                    bass.mybir.AluOpType.max)
                            else:
                                nc.vector.tensor_tensor(
                                    msg[:], im_ps[:], gB[:, jr, :],
                                    bass.mybir.AluOpType.add)
                                nc.vector.tensor_scalar(
                                    msg[:], msg[:], 0.0, None,
                                    bass.mybir.AluOpType.max)
                            if dump and k == 1 and j == 0:
                                nc.sync.dma_start(dbg["msg0"][:], msg[:])
                                nc.sync.dma_start(dbg["atT0"][:],
                                                  atT[:, 0, 0:128])
                            for m in range(3):
                                nc.tensor.matmul(
                                    at_ps[:, m * 128:(m + 1) * 128],
                                    msg[:, m * 128:(m + 1) * 128],
                                    Ssb[:, jr, :],
                                    start=(jj == 0 and m == 0),
                                    stop=(jj == nchk - 1 and m == 2),
                                    skip_group_check=True)
                        for m in range(3):
                            dstc = ATacc[:, m * NPC + t * 128: m * NPC + (t + 1) * 128]
                            nc.vector.tensor_tensor(
                                dstc, at_ps[:, m * 128:(m + 1) * 128], dstc,
                                bass.mybir.AluOpType.add)
                # tail
                if dump:
                    nc.sync.dma_start(dbg[f"at{k}"][:], ATacc[:])
                if k < 3:
                    Bout = B2 if k == 1 else B3
                    for t in range(TPB):
                        b_ps = ps_big.tile([128, 384], f32, tag="big")
                        for m in range(3):
                            lhs = ATacc[0:(128 if m < 2 else 64),
                                        m * NPC + t * 128: m * NPC + (t + 1) * 128]
                            nc.tensor.matmul(
                                b_ps[:], lhs, wht_s[m][:],
                                start=(m == 0), stop=(m == 2),
                                skip_group_check=True)
                        bsb = mpool.tile([128, 384], bf16, tag="msg")
                        nc.vector.tensor_copy(bsb[:], b_ps[:])
                        nc.sync.dma_start(Bout[t * 128:(t + 1) * 128, :], bsb[:])
                    if k == 1:
                        nc.gpsimd.collective_compute(
                            "AllGather", bass.mybir.AluOpType.bypass,
                            replica_groups=[list(range(C))],
                            ins=[B2.opt()], outs=[Bfull.opt()])
                    if dump:
                        if k == 1:
                            nc.sync.dma_start(dbg["b2"][:], B2[:])
                            nc.sync.dma_start(dbg["bfull"][:], Bfull[:])
                        else:
                            nc.sync.dma_start(dbg["b3"][:], B3[:])
                else:
                    # readout
                    acc = smallpool.tile([128, 3], f32, tag="acc")
                    nc.vector.memset(acc[:], 0.0)
                    for t in range(TPB):
                        atr = smallpool.tile([128, 128], bf16, tag="atr")
                        nc.sync.dma_start(atr[:], atomT_read[:, t * 128:(t + 1) * 128])
                        ia = []
                        for m in range(3):
                            ia_m = smallpool.tile([128, 128], bf16, tag=f"ia{m}")
                            ia.append(ia_m)
                            nc.vector.tensor_copy(
                                ia_m[:], ATacc[:, m * NPC + t * 128: m * NPC + (t + 1) * 128])
                        ar_ps = ps_big.tile([128, 384], f32, tag="big")
                        for m in range(3):
                            dstp = ar_ps[:, m * 128:(m + 1) * 128]
                            nc.tensor.matmul(dstp, wo_ka_s[:, m * 128:(m + 1) * 128],
                                             atr[:], start=(m == 0), stop=False,
                                             skip_group_check=True)
                            nc.tensor.matmul(dstp, wo_k2_s[:, m * 128:(m + 1) * 128],
                                             ia[0][:], start=False, stop=False,
                                             skip_group_check=True)
                            nc.tensor.matmul(dstp, wo_k3_s[:, m * 128:(m + 1) * 128],
                                             ia[1][:], start=False, stop=False,
                                             skip_group_check=True)
                            nc.tensor.matmul(dstp, wo_k4_s[:, m * 128:(m + 1) * 128],
                                             ia[2][0:64, :], start=False,
                                             stop=(m == 2), skip_group_check=True)
                        arsb = mpool.tile([128, 384], f32, tag="ar")
                        nc.vector.tensor_scalar(arsb[:], ar_ps[:], 0.0, None,
                                                bass.mybir.AluOpType.max)
                        red = smallpool.tile([128, 3], f32, tag="red")
                        for m in range(3):
                            nc.vector.reduce_sum(
                                red[:, m:m + 1], arsb[:, m * 128:(m + 1) * 128],
                                axis=bass.mybir.AxisListType.X)
                        nc.vector.tensor_tensor(acc[:], red[:], acc[:],
                                                bass.mybir.AluOpType.add)
                    accd = dram.tile([128, 3], f32)
                    accr_d = dram.tile([128, 3], f32)
                    accsb = smallpool.tile([128, 3], f32, tag="accr")
                    nc.sync.dma_start(accd[:], acc[:])
                    nc.gpsimd.collective_compute(
                        "AllReduce", bass.mybir.AluOpType.add,
                        replica_groups=[list(range(C))],
                        ins=[accd.opt()], outs=[accr_d.opt()])
                    nc.sync.dma_start(accsb[:], accr_d[:])
                    o_ps = ps_big.tile([1, 320], f32, tag="big")
                    for cc in range(3):
                        nc.tensor.matmul(o_ps[:], accsb[:, cc:cc + 1], woutt_s[cc][:],
                                         start=(cc == 0), stop=False,
                                         skip_group_check=True)
                    nc.tensor.matmul(o_ps[:], one_s[:], bout_s[:],
                                     start=False, stop=True, skip_group_check=True)
                    osb = smallpool.tile([1, 320], f32, tag="osb")
                    nc.vector.tensor_scalar(osb[:], o_ps[:], 0.0, None,
                                            bass.mybir.AluOpType.max)
                    nc.sync.dma_start(out_d[:], osb[:])

            sweep(1)
            sweep(2)
            sweep(3)

    nc.compile()
    return nc


_last_results = None


def kernel(**inputs):
    """Full-shape entry point: returns [300] float32."""
    global _last_results
    trace = bool(inputs.pop("_trace", False))
    atom = np.asarray(inputs["atom_features"], np.float32)
    ef = np.asarray(inputs["edge_features"], np.float32)
    src = np.asarray(inputs["edge_src"]).astype(np.int64)
    dst = np.asarray(inputs["edge_dst"]).astype(np.int64)
    args = [atom, ef, src, dst] + [np.asarray(inputs[k], np.float32) for k in
                                   ("Wi", "bi", "Wh", "bh", "Wo", "bo", "Wout", "bout")]
    meta, in_maps = preprocess(*args)
    nc = build_nc(meta)
    from concourse.bass_utils import run_bass_kernel_spmd
    res = run_bass_kernel_spmd(nc, in_maps, list(range(meta["C"])), trace=trace)
    _last_results = res
    out = np.asarray(res.results[0]["out"]).reshape(-1)[:H].astype(np.float32)
    return out



# revision 15
# speedup vs baseline: 2.9629x; 2.9629x over previous
"""DMPNN encoder on 8 TRN2 cores via Bass/Tile — v3.

Design (per core, node-sharded npc nodes/core):
  Host pre-gathers per-token feature tables F [128, TOK] (117 atom[dst] rows,
  10 edge-feature rows, const-1 row 127), so imsg is ONE matmul F.T @ WiC per
  128-token chunk (bi baked in row 127).
  Edge tokens are bucketed by (dst-row-quadrant, src tile) for sweeps 1-2 and
  by dst tile for sweep 3; A^T accumulates per-tile in [node, h] layout via a
  single S-stationary matmul per chunk; tails transpose A per tile on the PE
  (identity matmul) before the bf16 Wh matmuls.
  sweep1 (src order): msg1 = relu(imsg [+ has_nb*bh]); quadrant q of the B2
    table is AllGathered as soon as its 10 tiles are done (4 overlapped
    collectives into a Shared Bfull laid out quadrant-major).
  sweep2 (src order): per group dma_gather of Bfull[dst] rows (quadrant-
    sliced source, so quad-0 groups start right after the first AllGather);
    DVE adds the rows into imsg, ACT applies relu.
  sweep3 (dst order): B3 never leaves SBUF; its per-edge expansion is a
    one-hot matmul Gd.T @ B3_tile into the imsg PSUM; then relu, S_dst
    matmuls -> in_agg^T; readout (Wo with bo+mask baked, masked sums,
    AllReduce, Wout).

Biases: bi via const-1 feature row; bh via a const-1 column in the A^T
accumulator against an extra Wh row (only when bh != 0; zero-row/zero-col
tricks give has_nb masking). bo baked in atomT_read mask row. Self-loop
correction skipped (error ~5e-6).
"""
import numpy as np
import ml_dtypes

BF16 = ml_dtypes.bfloat16

NODE_F = 117
EDGE_F = 10
H = 300
HW = 320                      # padded hidden width for streams
DEPTH = 3
Q = 4                         # AllGather quadrants


# ---------------------------------------------------------------- host side

def _pack_idx(idx):
    """[TOK] int -> [128, TOK/16] int16 in dma_gather wrap layout."""
    idx = np.asarray(idx, np.int64)
    assert len(idx) % 16 == 0
    a = idx.reshape(-1, 16).T.astype(np.int16)
    assert (idx < 32768).all() and (idx >= 0).all()
    return np.tile(a, (8, 1))


def preprocess(atom, ef, src, dst, Wi, bi, Wh, bh, Wo, bo, Wout, bout, C=8, gblk=20):
    N, E = atom.shape[0], src.shape[0]
    assert N % C == 0
    npc = N // C
    TPB = npc // 128 + 1          # always >= 1 pad row per core
    NPC = TPB * 128
    GROWS = C * NPC
    RPQ = NPC // Q                # rows per core per quadrant
    QROWS = C * RPQ               # rows per quadrant slab of Bfull
    TPQ = TPB // Q                # tiles per quadrant
    assert RPQ % 128 == 0 and QROWS <= 32768
    ZR = npc                      # core-0-local zero row (pad), lives in quad 3
    ZRQ = 0 * RPQ + (ZR - 3 * RPQ)   # its index within the quad-3 slab

    deg_src = np.bincount(src, minlength=N)
    self_loop = src == dst
    has_nb = (deg_src[dst] - self_loop.astype(np.int64)) > 0
    deg_in = np.bincount(dst, minlength=N)
    HAS_BH = bool(np.any(bh))

    meta = dict(C=C, N=N, E=E, npc=npc, TPB=TPB, NPC=NPC, GROWS=GROWS,
                RPQ=RPQ, QROWS=QROWS, TPQ=TPQ, HAS_BH=HAS_BH, orders={})
    percore = [dict() for _ in range(C)]

    for c in range(C):
        lo = c * npc
        # masked transposed readout table with mask row 127 (bakes bo + deg_in mask)
        atr = np.zeros((128, NPC), BF16)
        msk = (deg_in[lo:lo + npc] > 0)
        atr[:NODE_F, :npc] = (atom[lo:lo + npc].T * msk[None, :]).astype(BF16)
        atr[127, :npc] = msk.astype(BF16)
        percore[c]["atomT_read"] = atr

    # ---- weights (shared, replicated)
    shared = {}
    wic = np.zeros((128, HW), BF16)
    wic[:NODE_F, :H] = Wi[:, :NODE_F].T.astype(BF16)
    wic[NODE_F:NODE_F + EDGE_F, :H] = Wi[:, NODE_F:].T.astype(BF16)
    wic[127, :H] = bi.astype(BF16)
    shared["wic"] = wic
    shared["ident"] = np.eye(128, dtype=BF16)
    if HAS_BH:
        shared["bh_row"] = np.pad(bh.astype(BF16), (0, HW - H))[None, :]
    wht = np.zeros((320, 384), BF16)
    wht[:H, :H] = Wh.T.astype(BF16)
    if HAS_BH:
        wht[H, :H] = bh.astype(BF16)   # const-col -> +bh (row 44 of wht2)
    shared["wht0"] = wht[0:128]
    shared["wht1"] = wht[128:256]
    shared["wht2"] = wht[256:320]
    wo = np.zeros((448, 384), BF16)   # K rows: 0..127 atom(+mask@127), 128.. in_agg
    wo[:NODE_F, :H] = Wo[:, :NODE_F].T.astype(BF16)
    wo[127, :H] = bo.astype(BF16)
    wo[128:128 + H, :H] = Wo[:, NODE_F:].T.astype(BF16)
    shared["wo_ka"] = wo[0:128]
    shared["wo_k2"] = wo[128:256]
    shared["wo_k3"] = wo[256:384]
    shared["wo_k4"] = wo[384:448]
    wout = np.zeros((384, 320), np.float32)
    wout[:H, :H] = (Wout.T / N).astype(np.float32)
    shared["woutt0"] = wout[0:128]
    shared["woutt1"] = wout[128:256]
    shared["woutt2"] = wout[256:384]
    shared["bout_row"] = np.pad(bout.astype(np.float32), (0, 20))[None, :]
    shared["one_t"] = np.ones((1, 1), np.float32)

    # ---- per-order token layouts
    for order in ("src", "dst"):
        key = src if order == "src" else dst
        owner = key // npc
        loc = key - owner * npc
        tile_of = loc // 128
        if order == "src":
            # quadrant of the dst row in the quadrant-major Bfull layout
            r_in = dst % npc
            half_of = np.where(has_nb, r_in // RPQ, 3)
            vA = np.where(has_nb,
                          (dst // npc) * RPQ + r_in - half_of * RPQ, ZRQ)
            halves = Q
            sort_sub = vA
        else:
            half_of = np.zeros(E, np.int64)
            halves = 1
            sort_sub = loc

        # vectorized per-(core, h, t) bucketing
        gid = (owner * halves + half_of) * TPB + tile_of
        counts = np.bincount(gid, minlength=C * halves * TPB).reshape(C, halves, TPB)
        n_chunks = -(-counts.max(axis=0) // 128)  # [halves, TPB]
        blk0 = np.zeros((halves, TPB), np.int64)
        acc = 0
        for h in range(halves):
            for t in range(TPB):
                blk0[h, t] = acc
                acc += n_chunks[h, t]
        TOTBLK = int(acc)
        TOK = TOTBLK * 128

        # gather-call groups: contiguous tiles within a half, ~gblk chunks each
        groups = []
        for h in range(halves):
            t = 0
            while t < TPB:
                t0, nb = t, 0
                while t < TPB and (nb == 0 or nb + n_chunks[h, t] <= gblk):
                    nb += n_chunks[h, t]
                    t += 1
                if nb:
                    groups.append(dict(h=h, t0=t0, t1=t, b0=int(blk0[h, t0]),
                                       b1=int(blk0[h, t - 1] + n_chunks[h, t - 1])))
        om = dict(halves=halves, n_chunks=n_chunks, blk0=blk0, TOTBLK=TOTBLK,
                  TOK=TOK, groups=groups)
        meta["orders"][order] = om

        # vectorized token assignment: sort edges by (gid, gather row) for DMA
        # locality; position within group + per-(h,t) chunk base gives slots
        ordr = np.argsort(gid * 16384 + sort_sub, kind="stable")
        sorted_gid = gid[ordr]
        grp_starts = np.searchsorted(sorted_gid, np.arange(C * halves * TPB))
        within = np.arange(E) - grp_starts[sorted_gid]
        base_tok = np.broadcast_to((blk0 * 128)[None], (C, halves, TPB)).reshape(-1)
        tok_sorted = base_tok[sorted_gid] + within
        tok = np.empty(E, np.int64)
        tok[ordr] = tok_sorted

        for c in range(C):
            sel = owner == c
            tk = tok[sel]
            F = np.zeros((128, TOK), BF16)
            F[:NODE_F, tk] = atom[dst[sel]].T.astype(BF16)
            F[NODE_F:NODE_F + EDGE_F, tk] = ef[sel].T.astype(BF16)
            F[127, tk] = 1.0
            percore[c][f"F_{order}"] = F
            S = np.zeros((128, TOTBLK, 128), BF16)
            ltile = loc[sel] - tile_of[sel] * 128
            S[tk % 128, tk // 128, ltile] = 1.0
            percore[c][f"S_{order}"] = S
            if order == "src":
                idxB = np.full(TOK, ZRQ, np.int64)
                idxB[tk] = vA[sel]
                percore[c]["idxB_src"] = _pack_idx(idxB)
                if HAS_BH:
                    hnb = np.zeros((1, TOK), BF16)
                    hnb[0, tk] = has_nb[sel].astype(BF16)
                    percore[c]["hnb_src"] = hnb
            else:
                Gd = np.zeros((128, TOTBLK, 128), BF16)
                Gd[ltile, tk // 128, tk % 128] = has_nb[sel].astype(BF16)
                percore[c]["Gd_dst"] = Gd

    in_maps = []
    for c in range(C):
        m = dict(shared)
        m.update(percore[c])
        in_maps.append(m)
    return meta, in_maps


# ---------------------------------------------------------------- device side

def build_nc(meta, debug=False):
    import concourse.bass as bass
    import concourse.tile as tile
    from concourse import bacc, mybir
    from concourse.library_config import mlp

    C, NPC, TPB = meta["C"], meta["NPC"], meta["TPB"]
    GROWS, npc = meta["GROWS"], meta["npc"]
    RPQ, QROWS, TPQ = meta["RPQ"], meta["QROWS"], meta["TPQ"]
    HAS_BH = meta["HAS_BH"]
    f32, bf16, i16 = mybir.dt.float32, mybir.dt.bfloat16, mybir.dt.int16
    RELU = mybir.ActivationFunctionType.Relu
    MAX = bass.mybir.AluOpType.max
    ADD = bass.mybir.AluOpType.add

    nc = bacc.Bacc("TRN2", target_bir_lowering=False, debug=debug, num_devices=C)

    def din(name, shape, dt):
        return nc.dram_tensor(name, shape, dt, kind="ExternalInput")

    oms = meta["orders"]
    atomT_read = din("atomT_read", [128, NPC], bf16)
    ins = {}
    for o in ("src", "dst"):
        om = oms[o]
        ins[f"F_{o}"] = din(f"F_{o}", [128, om["TOK"]], bf16)
        ins[f"S_{o}"] = din(f"S_{o}", [128, om["TOTBLK"], 128], bf16)
    ins["idxB_src"] = din("idxB_src", [128, oms["src"]["TOK"] // 16], i16)
    ins["Gd_dst"] = din("Gd_dst", [128, oms["dst"]["TOTBLK"], 128], bf16)
    if HAS_BH:
        ins["hnb_src"] = din("hnb_src", [1, oms["src"]["TOK"]], bf16)
        bh_row = din("bh_row", [1, HW], bf16)
    wic = din("wic", [128, HW], bf16)
    ident = din("ident", [128, 128], bf16)
    wht = [din(f"wht{i}", [128 if i < 2 else 64, 384], bf16) for i in range(3)]
    wo_ka = din("wo_ka", [128, 384], bf16)
    wo_k2 = din("wo_k2", [128, 384], bf16)
    wo_k3 = din("wo_k3", [128, 384], bf16)
    wo_k4 = din("wo_k4", [64, 384], bf16)
    woutt = [din(f"woutt{i}", [128, 320], f32) for i in range(3)]
    bout_row = din("bout_row", [1, 320], f32)
    one_t = din("one_t", [1, 1], f32)
    out_d = nc.dram_tensor("out", [1, 320], f32, kind="ExternalOutput")

    with tile.TileContext(nc) as tc:
        nc.gpsimd.load_library(mlp)
        import contextlib
        ctx = contextlib.ExitStack()
        with ctx:
            cpool = ctx.enter_context(tc.tile_pool(name="consts", bufs=1))
            idxpool = ctx.enter_context(tc.tile_pool(name="idx", bufs=1))
            fpool = ctx.enter_context(tc.tile_pool(name="F", bufs=2))
            spool = ctx.enter_context(tc.tile_pool(name="S", bufs=2))
            gpool = ctx.enter_context(tc.tile_pool(name="gB", bufs=4))
            gdpool = ctx.enter_context(tc.tile_pool(name="Gd", bufs=2))
            hpool = ctx.enter_context(tc.tile_pool(name="hnb", bufs=2))
            mpool = ctx.enter_context(tc.tile_pool(name="msg", bufs=3))
            trpool = ctx.enter_context(tc.tile_pool(name="tr", bufs=2))
            accpool = ctx.enter_context(tc.tile_pool(name="ATacc", bufs=1))
            b3pool = ctx.enter_context(tc.tile_pool(name="B3", bufs=1))
            smallpool = ctx.enter_context(tc.tile_pool(name="small", bufs=4))
            ps_big = ctx.enter_context(tc.tile_pool(name="ps_big", bufs=2, space="PSUM"))
            ps_at = ctx.enter_context(tc.tile_pool(name="ps_at", bufs=2, space="PSUM"))
            ps_tr = ctx.enter_context(tc.tile_pool(name="ps_tr", bufs=2, space="PSUM"))
            dram = ctx.enter_context(tc.tile_pool(name="dram", bufs=1, space="DRAM"))

            def cload(t, shape, dt):
                s = cpool.tile(shape, dt, tag=t.name)
                nc.sync.dma_start(s[:], t[:])
                return s

            wic_s = cload(wic, [128, HW], bf16)
            ident_s = cload(ident, [128, 128], bf16)
            if HAS_BH:
                bh_row_s = cload(bh_row, [1, HW], bf16)
            wht_s = [cload(w, [128 if i < 2 else 64, 384], bf16) for i, w in enumerate(wht)]
            wo_ka_s = cload(wo_ka, [128, 384], bf16)
            wo_k2_s = cload(wo_k2, [128, 384], bf16)
            wo_k3_s = cload(wo_k3, [128, 384], bf16)
            wo_k4_s = cload(wo_k4, [64, 384], bf16)
            woutt_s = [cload(w, [128, 320], f32) for w in woutt]
            bout_s = cload(bout_row, [1, 320], f32)
            one_s = cload(one_t, [1, 1], f32)

            B2 = dram.tile([NPC, 384], bf16)
            Bfull_q = [dram.tile([QROWS, 384], bf16, addr_space="Shared",
                                 name=f"Bfullq{q}") for q in range(Q)]
            B3_sb = b3pool.tile([128, TPB * HW], bf16, tag="B3sb")

            relu_flip = [0]

            def relu(dst_ap, src_ap):
                if relu_flip[0] % 2 == 0:
                    nc.scalar.activation(dst_ap, src_ap, RELU)
                else:
                    nc.vector.tensor_scalar(dst_ap, src_ap, 0.0, None, MAX)
                relu_flip[0] += 1

            def transpose_tile(ATacc, t):
                """ATacc [node, h] tile t -> trsb [h, m*128+node] bf16."""
                tr_ps = ps_tr.tile([128, 384], f32, tag="tr")
                for m in range(3):
                    nc.tensor.matmul(
                        tr_ps[:, m * 128:(m + 1) * 128],
                        ATacc[:, t * 384 + m * 128: t * 384 + (m + 1) * 128],
                        ident_s[:], start=(m == 0), stop=(m == 2),
                        skip_group_check=True)
                trsb = trpool.tile([128, 384], bf16, tag="trsb")
                nc.vector.tensor_copy(trsb[:], tr_ps[:])
                return trsb

            def sweep1():
                """Tile-major sweep 1: each tile's 4 quadrant buckets run
                back-to-back so its B2 tile (and each quadrant AllGather)
                fires progressively during the sweep."""
                om = oms["src"]
                blk0, n_chunks = om["blk0"], om["n_chunks"]
                ATacc = accpool.tile([128, TPB * 384], bf16, tag="ATacc")
                nc.vector.memset(ATacc[:], 0.0)
                TR = 5
                ag_done = [0]
                for tb in range(0, TPB, TR):
                    te = min(tb + TR, TPB)
                    first = [True] * (te - tb)
                    for q in range(Q):
                        b0 = int(blk0[q][tb])
                        b1 = int(blk0[q][te - 1] + n_chunks[q][te - 1])
                        nb = b1 - b0
                        if nb == 0:
                            continue
                        ntok = nb * 128
                        Fg = fpool.tile([128, ntok], bf16, tag="F")
                        nc.sync.dma_start(Fg[:], ins["F_src"][:, b0 * 128:b1 * 128])
                        Ssb = spool.tile([128, nb, 128], bf16, tag="S")
                        nc.sync.dma_start(Ssb[:], ins["S_src"][:, b0:b1, :])
                        hnb_g = None
                        if HAS_BH:
                            hnb_g = hpool.tile([1, ntok], bf16, tag="hnb")
                            nc.sync.dma_start(hnb_g[:],
                                              ins["hnb_src"][:, b0 * 128:b1 * 128])
                        for t in range(tb, te):
                            nchk = int(n_chunks[q][t])
                            if nchk == 0:
                                continue
                            at_ps = ps_at.tile([128, HW], f32, tag="at_ps")
                            jb0 = int(blk0[q][t])
                            for jj in range(nchk):
                                jr = jb0 + jj - b0
                                im_ps = ps_big.tile([128, HW], f32, tag="big")
                                nc.tensor.matmul(
                                    im_ps[:], Fg[:, jr * 128:(jr + 1) * 128],
                                    wic_s[:], start=True, stop=not HAS_BH,
                                    skip_group_check=True)
                                if HAS_BH:
                                    nc.tensor.matmul(
                                        im_ps[:], hnb_g[:, jr * 128:(jr + 1) * 128],
                                        bh_row_s[:], start=False, stop=True,
                                        skip_group_check=True)
                                msg = mpool.tile([128, HW], bf16, tag="msg")
                                relu(msg[:], im_ps[:])
                                nc.tensor.matmul(
                                    at_ps[:], Ssb[:, jr, :], msg[:],
                                    start=(jj == 0), stop=(jj == nchk - 1),
                                    skip_group_check=True)
                            dstc = ATacc[:, t * 384: t * 384 + HW]
                            if first[t - tb]:
                                nc.vector.tensor_copy(dstc, at_ps[:])
                                first[t - tb] = False
                            else:
                                nc.vector.tensor_tensor(dstc, at_ps[:], dstc, ADD)
                    for t in range(tb, te):
                        if HAS_BH:
                            nreal = min(npc - t * 128, 128)
                            if nreal > 0:
                                nc.vector.memset(
                                    ATacc[0:nreal, t * 384 + H: t * 384 + H + 1], 1.0)
                        trsb = transpose_tile(ATacc, t)
                        b_ps = ps_big.tile([128, 384], f32, tag="bps")
                        for m in range(3):
                            lhs = trsb[0:(128 if m < 2 else 64),
                                       m * 128:(m + 1) * 128]
                            nc.tensor.matmul(
                                b_ps[:], lhs, wht_s[m][:],
                                start=(m == 0), stop=(m == 2),
                                skip_group_check=True)
                        bsb = mpool.tile([128, 384], bf16, tag="bsb")
                        nc.vector.tensor_copy(bsb[:], b_ps[:])
                        nc.sync.dma_start(B2[t * 128:(t + 1) * 128, :], bsb[:])
                    while (ag_done[0] + 1) * TPQ <= te:
                        q = ag_done[0]
                        nc.gpsimd.collective_compute(
                            "AllGather", bass.mybir.AluOpType.bypass,
                            replica_groups=[list(range(C))],
                            ins=[B2[q * RPQ:(q + 1) * RPQ, :].opt()],
                            outs=[Bfull_q[q].opt()])
                        ag_done[0] += 1

            def sweep(k):
                order = "src" if k < 3 else "dst"
                om = oms[order]
                if k == 2:
                    idxB = idxpool.tile([128, om["TOK"] // 16], i16, tag="idxB")
                    nc.sync.dma_start(idxB[:], ins["idxB_src"][:])
                ATacc = accpool.tile([128, TPB * 384], bf16, tag="ATacc")
                nc.vector.memset(ATacc[:], 0.0)
                first = [True] * TPB
                for g in om["groups"]:
                    h, b0, b1 = g["h"], g["b0"], g["b1"]
                    nb = b1 - b0
                    ntok = nb * 128
                    Fg = fpool.tile([128, ntok], bf16, tag="F")
                    nc.sync.dma_start(Fg[:], ins[f"F_{order}"][:, b0 * 128:b1 * 128])
                    Ssb = spool.tile([128, nb, 128], bf16, tag="S")
                    nc.sync.dma_start(Ssb[:], ins[f"S_{order}"][:, b0:b1, :])
                    hnb_g = gB = Gd_g = None
                    if k == 1 and HAS_BH:
                        hnb_g = hpool.tile([1, ntok], bf16, tag="hnb")
                        nc.sync.dma_start(hnb_g[:], ins["hnb_src"][:, b0 * 128:b1 * 128])
                    if k == 2:
                        gB = gpool.tile([128, nb, 384], bf16, tag="gB")
                        nc.gpsimd.dma_gather(
                            gB[:], Bfull_q[h][:],
                            idxB[:, b0 * 8:b0 * 8 + ntok // 16],
                            ntok, ntok, 384, single_packet=False)
                    if k == 3:
                        Gd_g = gdpool.tile([128, nb, 128], bf16, tag="Gd")
                        nc.sync.dma_start(Gd_g[:], ins["Gd_dst"][:, b0:b1, :])
                    for t in range(g["t0"], g["t1"]):
                        nchk = int(om["n_chunks"][h][t])
                        if nchk == 0:
                            continue
                        at_ps = ps_at.tile([128, HW], f32, tag="at_ps")
                        jb0 = int(om["blk0"][h][t])
                        for jj in range(nchk):
                            j = jb0 + jj
                            jr = j - b0
                            im_ps = ps_big.tile([128, HW], f32, tag="big")
                            one_mm = k == 1 and not HAS_BH or k == 2
                            nc.tensor.matmul(
                                im_ps[:], Fg[:, jr * 128:(jr + 1) * 128],
                                wic_s[:], start=True, stop=one_mm,
                                skip_group_check=True)
                            if k == 1 and HAS_BH:
                                nc.tensor.matmul(
                                    im_ps[:], hnb_g[:, jr * 128:(jr + 1) * 128],
                                    bh_row_s[:], start=False, stop=True,
                                    skip_group_check=True)
                            elif k == 3:
                                nc.tensor.matmul(
                                    im_ps[:], Gd_g[:, jr, :],
                                    B3_sb[:, t * HW:(t + 1) * HW],
                                    start=False, stop=True, skip_group_check=True)
                            msg = mpool.tile([128, HW], bf16, tag="msg")
                            if k == 2:
                                nc.vector.tensor_tensor(
                                    msg[:], im_ps[:], gB[:, jr, 0:HW], ADD)
                                nc.scalar.activation(msg[:], msg[:], RELU)
                            else:
                                relu(msg[:], im_ps[:])
                            nc.tensor.matmul(
                                at_ps[:], Ssb[:, jr, :], msg[:],
                                start=(jj == 0), stop=(jj == nchk - 1),
                                skip_group_check=True)
                        dstc = ATacc[:, t * 384: t * 384 + HW]
                        if first[t]:
                            nc.vector.tensor_copy(dstc, at_ps[:])
                            first[t] = False
                        else:
                            nc.vector.tensor_tensor(dstc, at_ps[:], dstc, ADD)
                # tail
                if k < 3:
                    if HAS_BH:
                        for t in range(TPB):
                            nreal = min(npc - t * 128, 128)
                            if nreal > 0:
                                nc.vector.memset(
                                    ATacc[0:nreal, t * 384 + H: t * 384 + H + 1], 1.0)
                    for t in range(TPB):
                        trsb = transpose_tile(ATacc, t)
                        b_ps = ps_big.tile([128, 384], f32, tag="bps")
                        for m in range(3):
                            lhs = trsb[0:(128 if m < 2 else 64),
                                       m * 128:(m + 1) * 128]
                            nc.tensor.matmul(
                                b_ps[:], lhs, wht_s[m][:],
                                start=(m == 0), stop=(m == 2),
                                skip_group_check=True)
                        if k == 1:
                            bsb = mpool.tile([128, 384], bf16, tag="bsb")
                            nc.vector.tensor_copy(bsb[:], b_ps[:])
                            nc.sync.dma_start(B2[t * 128:(t + 1) * 128, :], bsb[:])
                            if t % TPQ == TPQ - 1:
                                q = t // TPQ
                                nc.gpsimd.collective_compute(
                                    "AllGather", bass.mybir.AluOpType.bypass,
                                    replica_groups=[list(range(C))],
                                    ins=[B2[q * RPQ:(q + 1) * RPQ, :].opt()],
                                    outs=[Bfull_q[q].opt()])
                        else:
                            nc.vector.tensor_copy(
                                B3_sb[:, t * HW:(t + 1) * HW], b_ps[:, 0:HW])
                else:
                    # readout
                    acc = smallpool.tile([128, 3], f32, tag="acc")
                    nc.vector.memset(acc[:], 0.0)
                    for t in range(TPB):
                        atr = smallpool.tile([128, 128], bf16, tag="atr")
                        nc.sync.dma_start(atr[:], atomT_read[:, t * 128:(t + 1) * 128])
                        trsb = transpose_tile(ATacc, t)
                        ar_ps = ps_big.tile([128, 384], f32, tag="bps")
                        for m in range(3):
                            dstp = ar_ps[:, m * 128:(m + 1) * 128]
                            nc.tensor.matmul(dstp, wo_ka_s[:, m * 128:(m + 1) * 128],
                                             atr[:], start=(m == 0), stop=False,
                                             skip_group_check=True)
                            nc.tensor.matmul(dstp, wo_k2_s[:, m * 128:(m + 1) * 128],
                                             trsb[:, 0:128], start=False, stop=False,
                                             skip_group_check=True)
                            nc.tensor.matmul(dstp, wo_k3_s[:, m * 128:(m + 1) * 128],
                                             trsb[:, 128:256], start=False, stop=False,
                                             skip_group_check=True)
                            nc.tensor.matmul(dstp, wo_k4_s[:, m * 128:(m + 1) * 128],
                                             trsb[0:64, 256:384], start=False,
                                             stop=(m == 2), skip_group_check=True)
                        arsb = mpool.tile([128, 384], f32, tag="ar")
                        nc.vector.tensor_scalar(arsb[:], ar_ps[:], 0.0, None, MAX)
                        red = smallpool.tile([128, 3], f32, tag="red")
                        for m in range(3):
                            nc.vector.reduce_sum(
                                red[:, m:m + 1], arsb[:, m * 128:(m + 1) * 128],
                                axis=bass.mybir.AxisListType.X)
                        nc.vector.tensor_tensor(acc[:], red[:], acc[:], ADD)
                    accd = dram.tile([128, 3], f32)
                    accr_d = dram.tile([128, 3], f32)
                    accsb = smallpool.tile([128, 3], f32, tag="accr")
                    nc.sync.dma_start(accd[:], acc[:])
                    nc.gpsimd.collective_compute(
                        "AllReduce", bass.mybir.AluOpType.add,
                        replica_groups=[list(range(C))],
                        ins=[accd.opt()], outs=[accr_d.opt()])
                    nc.sync.dma_start(accsb[:], accr_d[:])
                    o_ps = ps_big.tile([1, 320], f32, tag="bps")
                    for cc in range(3):
                        nc.tensor.matmul(o_ps[:], accsb[:, cc:cc + 1], woutt_s[cc][:],
                                         start=(cc == 0), stop=False,
                                         skip_group_check=True)
                    nc.tensor.matmul(o_ps[:], one_s[:], bout_s[:],
                                     start=False, stop=True, skip_group_check=True)
                    osb = smallpool.tile([1, 320], f32, tag="osb")
                    nc.vector.tensor_scalar(osb[:], o_ps[:], 0.0, None, MAX)
                    nc.sync.dma_start(out_d[:], osb[:])

            sweep1()
            sweep(2)
            sweep(3)

    nc.compile()
    return nc


_last_results = None


def kernel(**inputs):
    """Full-shape entry point: returns [300] float32."""
    global _last_results
    trace = bool(inputs.pop("_trace", False))
    atom = np.asarray(inputs["atom_features"], np.float32)
    ef = np.asarray(inputs["edge_features"], np.float32)
    src = np.asarray(inputs["edge_src"]).astype(np.int64)
    dst = np.asarray(inputs["edge_dst"]).astype(np.int64)
    args = [atom, ef, src, dst] + [np.asarray(inputs[k], np.float32) for k in
                                   ("Wi", "bi", "Wh", "bh", "Wo", "bo", "Wout", "bout")]
    meta, in_maps = preprocess(*args)
    nc = build_nc(meta)
    from concourse.bass_utils import run_bass_kernel_spmd
    res = run_bass_kernel_spmd(nc, in_maps, list(range(meta["C"])), trace=trace)
    _last_results = res
    out = np.asarray(res.results[0]["out"]).reshape(-1)[:H].astype(np.float32)
    return out


# revision 16
# speedup vs baseline: 3.0512x; 1.0298x over previous
"""DMPNN encoder on 8 TRN2 cores via Bass/Tile — v3.

Design (per core, node-sharded npc nodes/core):
  Host pre-gathers per-token feature tables F [128, TOK] (117 atom[dst] rows,
  10 edge-feature rows, const-1 row 127), so imsg is ONE matmul F.T @ WiC per
  128-token chunk (bi baked in row 127).
  Edge tokens are bucketed by (dst-row-quadrant, src tile) for sweeps 1-2 and
  by dst tile for sweep 3; A^T accumulates per-tile in [node, h] layout via a
  single S-stationary matmul per chunk; tails transpose A per tile on the PE
  (identity matmul) before the bf16 Wh matmuls.
  sweep1 (src order): msg1 = relu(imsg [+ has_nb*bh]); quadrant q of the B2
    table is AllGathered as soon as its 10 tiles are done (4 overlapped
    collectives into a Shared Bfull laid out quadrant-major).
  sweep2 (src order): per group dma_gather of Bfull[dst] rows (quadrant-
    sliced source, so quad-0 groups start right after the first AllGather);
    DVE adds the rows into imsg, ACT applies relu.
  sweep3 (dst order): B3 never leaves SBUF; its per-edge expansion is a
    one-hot matmul Gd.T @ B3_tile into the imsg PSUM; then relu, S_dst
    matmuls -> in_agg^T; readout (Wo with bo+mask baked, masked sums,
    AllReduce, Wout).

Biases: bi via const-1 feature row; bh via a const-1 column in the A^T
accumulator against an extra Wh row (only when bh != 0; zero-row/zero-col
tricks give has_nb masking). bo baked in atomT_read mask row. Self-loop
correction skipped (error ~5e-6).
"""
import numpy as np
import ml_dtypes

BF16 = ml_dtypes.bfloat16

NODE_F = 117
EDGE_F = 10
H = 300
HW = 320                      # padded hidden width for streams
DEPTH = 3
Q = 4                         # AllGather quadrants


# ---------------------------------------------------------------- host side

def _pack_idx(idx):
    """[TOK] int -> [128, TOK/16] int16 in dma_gather wrap layout."""
    idx = np.asarray(idx, np.int64)
    assert len(idx) % 16 == 0
    a = idx.reshape(-1, 16).T.astype(np.int16)
    assert (idx < 32768).all() and (idx >= 0).all()
    return np.tile(a, (8, 1))


def preprocess(atom, ef, src, dst, Wi, bi, Wh, bh, Wo, bo, Wout, bout, C=8, gblk=20):
    N, E = atom.shape[0], src.shape[0]
    assert N % C == 0
    npc = N // C
    TPB = npc // 128 + 1          # always >= 1 pad row per core
    NPC = TPB * 128
    GROWS = C * NPC
    RPQ = NPC // Q                # rows per core per quadrant
    QROWS = C * RPQ               # rows per quadrant slab of Bfull
    TPQ = TPB // Q                # tiles per quadrant
    assert RPQ % 128 == 0 and QROWS <= 32768
    ZR = npc                      # core-0-local zero row (pad), lives in quad 3
    ZRQ = 0 * RPQ + (ZR - 3 * RPQ)   # its index within the quad-3 slab

    deg_src = np.bincount(src, minlength=N)
    self_loop = src == dst
    has_nb = (deg_src[dst] - self_loop.astype(np.int64)) > 0
    deg_in = np.bincount(dst, minlength=N)
    HAS_BH = bool(np.any(bh))

    meta = dict(C=C, N=N, E=E, npc=npc, TPB=TPB, NPC=NPC, GROWS=GROWS,
                RPQ=RPQ, QROWS=QROWS, TPQ=TPQ, HAS_BH=HAS_BH, orders={})
    percore = [dict() for _ in range(C)]

    for c in range(C):
        lo = c * npc
        # masked transposed readout table with mask row 127 (bakes bo + deg_in mask)
        atr = np.zeros((128, NPC), BF16)
        msk = (deg_in[lo:lo + npc] > 0)
        atr[:NODE_F, :npc] = (atom[lo:lo + npc].T * msk[None, :]).astype(BF16)
        atr[127, :npc] = msk.astype(BF16)
        percore[c]["atomT_read"] = atr

    # ---- weights (shared, replicated)
    shared = {}
    wic = np.zeros((128, HW), BF16)
    wic[:NODE_F, :H] = Wi[:, :NODE_F].T.astype(BF16)
    wic[NODE_F:NODE_F + EDGE_F, :H] = Wi[:, NODE_F:].T.astype(BF16)
    wic[127, :H] = bi.astype(BF16)
    shared["wic"] = wic
    shared["ident"] = np.eye(128, dtype=BF16)
    if HAS_BH:
        shared["bh_row"] = np.pad(bh.astype(BF16), (0, HW - H))[None, :]
    wht = np.zeros((320, 384), BF16)
    wht[:H, :H] = Wh.T.astype(BF16)
    if HAS_BH:
        wht[H, :H] = bh.astype(BF16)   # const-col -> +bh (row 44 of wht2)
    shared["wht0"] = wht[0:128]
    shared["wht1"] = wht[128:256]
    shared["wht2"] = wht[256:320]
    wo = np.zeros((448, 384), BF16)   # K rows: 0..127 atom(+mask@127), 128.. in_agg
    wo[:NODE_F, :H] = Wo[:, :NODE_F].T.astype(BF16)
    wo[127, :H] = bo.astype(BF16)
    wo[128:128 + H, :H] = Wo[:, NODE_F:].T.astype(BF16)
    shared["wo_ka"] = wo[0:128]
    shared["wo_k2"] = wo[128:256]
    shared["wo_k3"] = wo[256:384]
    shared["wo_k4"] = wo[384:448]
    wout = np.zeros((384, 320), np.float32)
    wout[:H, :H] = (Wout.T / N).astype(np.float32)
    shared["woutt0"] = wout[0:128]
    shared["woutt1"] = wout[128:256]
    shared["woutt2"] = wout[256:384]
    shared["bout_row"] = np.pad(bout.astype(np.float32), (0, 20))[None, :]
    shared["one_t"] = np.ones((1, 1), np.float32)

    # ---- per-order token layouts
    for order in ("src", "dst"):
        key = src if order == "src" else dst
        owner = key // npc
        loc = key - owner * npc
        tile_of = loc // 128
        if order == "src":
            # quadrant of the dst row in the quadrant-major Bfull layout
            r_in = dst % npc
            half_of = np.where(has_nb, r_in // RPQ, 3)
            vA = np.where(has_nb,
                          (dst // npc) * RPQ + r_in - half_of * RPQ, ZRQ)
            halves = Q
            sort_sub = vA
        else:
            half_of = np.zeros(E, np.int64)
            halves = 1
            sort_sub = loc

        # vectorized per-(core, h, t) bucketing
        gid = (owner * halves + half_of) * TPB + tile_of
        counts = np.bincount(gid, minlength=C * halves * TPB).reshape(C, halves, TPB)
        n_chunks = -(-counts.max(axis=0) // 128)  # [halves, TPB]
        blk0 = np.zeros((halves, TPB), np.int64)
        acc = 0
        for h in range(halves):
            for t in range(TPB):
                blk0[h, t] = acc
                acc += n_chunks[h, t]
        TOTBLK = int(acc)
        TOK = TOTBLK * 128

        # gather-call groups: contiguous tiles within a half, ~gblk chunks each
        groups = []
        for h in range(halves):
            t = 0
            while t < TPB:
                t0, nb = t, 0
                while t < TPB and (nb == 0 or nb + n_chunks[h, t] <= gblk):
                    nb += n_chunks[h, t]
                    t += 1
                if nb:
                    groups.append(dict(h=h, t0=t0, t1=t, b0=int(blk0[h, t0]),
                                       b1=int(blk0[h, t - 1] + n_chunks[h, t - 1])))
        om = dict(halves=halves, n_chunks=n_chunks, blk0=blk0, TOTBLK=TOTBLK,
                  TOK=TOK, groups=groups)
        meta["orders"][order] = om

        # vectorized token assignment: sort edges by (gid, gather row) for DMA
        # locality; position within group + per-(h,t) chunk base gives slots
        ordr = np.argsort(gid * 16384 + sort_sub, kind="stable")
        sorted_gid = gid[ordr]
        grp_starts = np.searchsorted(sorted_gid, np.arange(C * halves * TPB))
        within = np.arange(E) - grp_starts[sorted_gid]
        base_tok = np.broadcast_to((blk0 * 128)[None], (C, halves, TPB)).reshape(-1)
        tok_sorted = base_tok[sorted_gid] + within
        tok = np.empty(E, np.int64)
        tok[ordr] = tok_sorted

        for c in range(C):
            sel = owner == c
            tk = tok[sel]
            F = np.zeros((128, TOK), BF16)
            F[:NODE_F, tk] = atom[dst[sel]].T.astype(BF16)
            F[NODE_F:NODE_F + EDGE_F, tk] = ef[sel].T.astype(BF16)
            F[127, tk] = 1.0
            percore[c][f"F_{order}"] = F
            S = np.zeros((128, TOTBLK, 128), BF16)
            ltile = loc[sel] - tile_of[sel] * 128
            S[tk % 128, tk // 128, ltile] = 1.0
            percore[c][f"S_{order}"] = S
            if order == "src":
                idxB = np.full(TOK, ZRQ, np.int64)
                idxB[tk] = vA[sel]
                percore[c]["idxB_src"] = _pack_idx(idxB)
                if HAS_BH:
                    hnb = np.zeros((1, TOK), BF16)
                    hnb[0, tk] = has_nb[sel].astype(BF16)
                    percore[c]["hnb_src"] = hnb
            else:
                Gd = np.zeros((128, TOTBLK, 128), BF16)
                Gd[ltile, tk // 128, tk % 128] = has_nb[sel].astype(BF16)
                percore[c]["Gd_dst"] = Gd

    in_maps = []
    for c in range(C):
        m = dict(shared)
        m.update(percore[c])
        in_maps.append(m)
    return meta, in_maps


# ---------------------------------------------------------------- device side

def build_nc(meta, debug=False):
    import concourse.bass as bass
    import concourse.tile as tile
    from concourse import bacc, mybir
    from concourse.library_config import mlp

    C, NPC, TPB = meta["C"], meta["NPC"], meta["TPB"]
    GROWS, npc = meta["GROWS"], meta["npc"]
    RPQ, QROWS, TPQ = meta["RPQ"], meta["QROWS"], meta["TPQ"]
    HAS_BH = meta["HAS_BH"]
    f32, bf16, i16 = mybir.dt.float32, mybir.dt.bfloat16, mybir.dt.int16
    RELU = mybir.ActivationFunctionType.Relu
    MAX = bass.mybir.AluOpType.max
    ADD = bass.mybir.AluOpType.add

    nc = bacc.Bacc("TRN2", target_bir_lowering=False, debug=debug, num_devices=C)

    def din(name, shape, dt):
        return nc.dram_tensor(name, shape, dt, kind="ExternalInput")

    oms = meta["orders"]
    atomT_read = din("atomT_read", [128, NPC], bf16)
    ins = {}
    for o in ("src", "dst"):
        om = oms[o]
        ins[f"F_{o}"] = din(f"F_{o}", [128, om["TOK"]], bf16)
        ins[f"S_{o}"] = din(f"S_{o}", [128, om["TOTBLK"], 128], bf16)
    ins["idxB_src"] = din("idxB_src", [128, oms["src"]["TOK"] // 16], i16)
    ins["Gd_dst"] = din("Gd_dst", [128, oms["dst"]["TOTBLK"], 128], bf16)
    if HAS_BH:
        ins["hnb_src"] = din("hnb_src", [1, oms["src"]["TOK"]], bf16)
        bh_row = din("bh_row", [1, HW], bf16)
    wic = din("wic", [128, HW], bf16)
    ident = din("ident", [128, 128], bf16)
    wht = [din(f"wht{i}", [128 if i < 2 else 64, 384], bf16) for i in range(3)]
    wo_ka = din("wo_ka", [128, 384], bf16)
    wo_k2 = din("wo_k2", [128, 384], bf16)
    wo_k3 = din("wo_k3", [128, 384], bf16)
    wo_k4 = din("wo_k4", [64, 384], bf16)
    woutt = [din(f"woutt{i}", [128, 320], f32) for i in range(3)]
    bout_row = din("bout_row", [1, 320], f32)
    one_t = din("one_t", [1, 1], f32)
    out_d = nc.dram_tensor("out", [1, 320], f32, kind="ExternalOutput")

    with tile.TileContext(nc) as tc:
        nc.gpsimd.load_library(mlp)
        import contextlib
        ctx = contextlib.ExitStack()
        with ctx:
            cpool = ctx.enter_context(tc.tile_pool(name="consts", bufs=1))
            idxpool = ctx.enter_context(tc.tile_pool(name="idx", bufs=1))
            fpool = ctx.enter_context(tc.tile_pool(name="F", bufs=2))
            spool = ctx.enter_context(tc.tile_pool(name="S", bufs=2))
            gpool = ctx.enter_context(tc.tile_pool(name="gB", bufs=5))
            gdpool = ctx.enter_context(tc.tile_pool(name="Gd", bufs=2))
            hpool = ctx.enter_context(tc.tile_pool(name="hnb", bufs=2))
            mpool = ctx.enter_context(tc.tile_pool(name="msg", bufs=3))
            trpool = ctx.enter_context(tc.tile_pool(name="tr", bufs=2))
            accpool = ctx.enter_context(tc.tile_pool(name="ATacc", bufs=1))
            b3pool = ctx.enter_context(tc.tile_pool(name="B3", bufs=1))
            smallpool = ctx.enter_context(tc.tile_pool(name="small", bufs=4))
            ps_big = ctx.enter_context(tc.tile_pool(name="ps_big", bufs=2, space="PSUM"))
            ps_at = ctx.enter_context(tc.tile_pool(name="ps_at", bufs=2, space="PSUM"))
            ps_tr = ctx.enter_context(tc.tile_pool(name="ps_tr", bufs=2, space="PSUM"))
            dram = ctx.enter_context(tc.tile_pool(name="dram", bufs=1, space="DRAM"))

            def cload(t, shape, dt):
                s = cpool.tile(shape, dt, tag=t.name)
                nc.sync.dma_start(s[:], t[:])
                return s

            wic_s = cload(wic, [128, HW], bf16)
            ident_s = cload(ident, [128, 128], bf16)
            if HAS_BH:
                bh_row_s = cload(bh_row, [1, HW], bf16)
            wht_s = [cload(w, [128 if i < 2 else 64, 384], bf16) for i, w in enumerate(wht)]
            wo_ka_s = cload(wo_ka, [128, 384], bf16)
            wo_k2_s = cload(wo_k2, [128, 384], bf16)
            wo_k3_s = cload(wo_k3, [128, 384], bf16)
            wo_k4_s = cload(wo_k4, [64, 384], bf16)
            woutt_s = [cload(w, [128, 320], f32) for w in woutt]
            bout_s = cload(bout_row, [1, 320], f32)
            one_s = cload(one_t, [1, 1], f32)

            B2 = dram.tile([NPC, 384], bf16)
            Bfull_q = [dram.tile([QROWS, 384], bf16, addr_space="Shared",
                                 name=f"Bfullq{q}") for q in range(Q)]
            B3_sb = b3pool.tile([128, TPB * HW], bf16, tag="B3sb")

            relu_flip = [0]

            def relu(dst_ap, src_ap):
                if relu_flip[0] % 2 == 0:
                    nc.scalar.activation(dst_ap, src_ap, RELU)
                else:
                    nc.vector.tensor_scalar(dst_ap, src_ap, 0.0, None, MAX)
                relu_flip[0] += 1

            def transpose_tile(ATacc, t):
                """ATacc [node, h] tile t -> trsb [h, m*128+node] bf16."""
                tr_ps = ps_tr.tile([128, 384], f32, tag="tr")
                for m in range(3):
                    nc.tensor.matmul(
                        tr_ps[:, m * 128:(m + 1) * 128],
                        ATacc[:, t * 384 + m * 128: t * 384 + (m + 1) * 128],
                        ident_s[:], start=(m == 0), stop=(m == 2),
                        skip_group_check=True)
                trsb = trpool.tile([128, 384], bf16, tag="trsb")
                nc.vector.tensor_copy(trsb[:], tr_ps[:])
                return trsb

            def sweep1():
                """Tile-major sweep 1: each tile's 4 quadrant buckets run
                back-to-back so its B2 tile (and each quadrant AllGather)
                fires progressively during the sweep."""
                om = oms["src"]
                blk0, n_chunks = om["blk0"], om["n_chunks"]
                ATacc = accpool.tile([128, TPB * 384], bf16, tag="ATacc")
                nc.vector.memset(ATacc[:], 0.0)
                TR = 5
                ag_done = [0]
                for tb in range(0, TPB, TR):
                    te = min(tb + TR, TPB)
                    first = [True] * (te - tb)
                    for q in range(Q):
                        b0 = int(blk0[q][tb])
                        b1 = int(blk0[q][te - 1] + n_chunks[q][te - 1])
                        nb = b1 - b0
                        if nb == 0:
                            continue
                        ntok = nb * 128
                        Fg = fpool.tile([128, ntok], bf16, tag="F")
                        nc.sync.dma_start(Fg[:], ins["F_src"][:, b0 * 128:b1 * 128])
                        Ssb = spool.tile([128, nb, 128], bf16, tag="S")
                        nc.sync.dma_start(Ssb[:], ins["S_src"][:, b0:b1, :])
                        hnb_g = None
                        if HAS_BH:
                            hnb_g = hpool.tile([1, ntok], bf16, tag="hnb")
                            nc.sync.dma_start(hnb_g[:],
                                              ins["hnb_src"][:, b0 * 128:b1 * 128])
                        for t in range(tb, te):
                            nchk = int(n_chunks[q][t])
                            if nchk == 0:
                                continue
                            at_ps = ps_at.tile([128, HW], f32, tag="at_ps")
                            jb0 = int(blk0[q][t])
                            for jj in range(nchk):
                                jr = jb0 + jj - b0
                                im_ps = ps_big.tile([128, HW], f32, tag="big")
                                nc.tensor.matmul(
                                    im_ps[:], Fg[:, jr * 128:(jr + 1) * 128],
                                    wic_s[:], start=True, stop=not HAS_BH,
                                    skip_group_check=True)
                                if HAS_BH:
                                    nc.tensor.matmul(
                                        im_ps[:], hnb_g[:, jr * 128:(jr + 1) * 128],
                                        bh_row_s[:], start=False, stop=True,
                                        skip_group_check=True)
                                msg = mpool.tile([128, HW], bf16, tag="msg")
                                relu(msg[:], im_ps[:])
                                nc.tensor.matmul(
                                    at_ps[:], Ssb[:, jr, :], msg[:],
                                    start=(jj == 0), stop=(jj == nchk - 1),
                                    skip_group_check=True)
                            dstc = ATacc[:, t * 384: t * 384 + HW]
                            if first[t - tb]:
                                nc.vector.tensor_copy(dstc, at_ps[:])
                                first[t - tb] = False
                            else:
                                nc.vector.tensor_tensor(dstc, at_ps[:], dstc, ADD)
                    for t in range(tb, te):
                        if HAS_BH:
                            nreal = min(npc - t * 128, 128)
                            if nreal > 0:
                                nc.vector.memset(
                                    ATacc[0:nreal, t * 384 + H: t * 384 + H + 1], 1.0)
                        trsb = transpose_tile(ATacc, t)
                        b_ps = ps_big.tile([128, 384], f32, tag="bps")
                        for m in range(3):
                            lhs = trsb[0:(128 if m < 2 else 64),
                                       m * 128:(m + 1) * 128]
                            nc.tensor.matmul(
                                b_ps[:], lhs, wht_s[m][:],
                                start=(m == 0), stop=(m == 2),
                                skip_group_check=True)
                        bsb = mpool.tile([128, 384], bf16, tag="bsb")
                        nc.vector.tensor_copy(bsb[:], b_ps[:])
                        nc.sync.dma_start(B2[t * 128:(t + 1) * 128, :], bsb[:])
                    while (ag_done[0] + 1) * TPQ <= te:
                        q = ag_done[0]
                        nc.gpsimd.collective_compute(
                            "AllGather", bass.mybir.AluOpType.bypass,
                            replica_groups=[list(range(C))],
                            ins=[B2[q * RPQ:(q + 1) * RPQ, :].opt()],
                            outs=[Bfull_q[q].opt()])
                        ag_done[0] += 1

            def sweep(k):
                order = "src" if k < 3 else "dst"
                om = oms[order]
                if k == 2:
                    idxB = idxpool.tile([128, om["TOK"] // 16], i16, tag="idxB")
                    nc.sync.dma_start(idxB[:], ins["idxB_src"][:])
                ATacc = accpool.tile([128, TPB * 384], bf16, tag="ATacc")
                nc.vector.memset(ATacc[:], 0.0)
                first = [True] * TPB
                for g in om["groups"]:
                    h, b0, b1 = g["h"], g["b0"], g["b1"]
                    nb = b1 - b0
                    ntok = nb * 128
                    Fg = fpool.tile([128, ntok], bf16, tag="F")
                    nc.sync.dma_start(Fg[:], ins[f"F_{order}"][:, b0 * 128:b1 * 128])
                    Ssb = spool.tile([128, nb, 128], bf16, tag="S")
                    nc.sync.dma_start(Ssb[:], ins[f"S_{order}"][:, b0:b1, :])
                    hnb_g = gB = Gd_g = None
                    if k == 1 and HAS_BH:
                        hnb_g = hpool.tile([1, ntok], bf16, tag="hnb")
                        nc.sync.dma_start(hnb_g[:], ins["hnb_src"][:, b0 * 128:b1 * 128])
                    if k == 2:
                        gB = gpool.tile([128, nb, 384], bf16, tag="gB")
                        nc.gpsimd.dma_gather(
                            gB[:], Bfull_q[h][:],
                            idxB[:, b0 * 8:b0 * 8 + ntok // 16],
                            ntok, ntok, 384, single_packet=False)
                    if k == 3:
                        Gd_g = gdpool.tile([128, nb, 128], bf16, tag="Gd")
                        nc.sync.dma_start(Gd_g[:], ins["Gd_dst"][:, b0:b1, :])
                    for t in range(g["t0"], g["t1"]):
                        nchk = int(om["n_chunks"][h][t])
                        if nchk == 0:
                            continue
                        at_ps = ps_at.tile([128, HW], f32, tag="at_ps")
                        jb0 = int(om["blk0"][h][t])
                        for jj in range(nchk):
                            j = jb0 + jj
                            jr = j - b0
                            im_ps = ps_big.tile([128, HW], f32, tag="big")
                            one_mm = k == 1 and not HAS_BH or k == 2
                            nc.tensor.matmul(
                                im_ps[:], Fg[:, jr * 128:(jr + 1) * 128],
                                wic_s[:], start=True, stop=one_mm,
                                skip_group_check=True)
                            if k == 1 and HAS_BH:
                                nc.tensor.matmul(
                                    im_ps[:], hnb_g[:, jr * 128:(jr + 1) * 128],
                                    bh_row_s[:], start=False, stop=True,
                                    skip_group_check=True)
                            elif k == 3:
                                nc.tensor.matmul(
                                    im_ps[:], Gd_g[:, jr, :],
                                    B3_sb[:, t * HW:(t + 1) * HW],
                                    start=False, stop=True, skip_group_check=True)
                            msg = mpool.tile([128, HW], bf16, tag="msg")
                            if k == 2:
                                nc.vector.tensor_tensor(
                                    msg[:], im_ps[:], gB[:, jr, 0:HW], ADD)
                                nc.scalar.activation(msg[:], msg[:], RELU)
                            else:
                                relu(msg[:], im_ps[:])
                            nc.tensor.matmul(
                                at_ps[:], Ssb[:, jr, :], msg[:],
                                start=(jj == 0), stop=(jj == nchk - 1),
                                skip_group_check=True)
                        dstc = ATacc[:, t * 384: t * 384 + HW]
                        if first[t]:
                            nc.vector.tensor_copy(dstc, at_ps[:])
                            first[t] = False
                        else:
                            nc.vector.tensor_tensor(dstc, at_ps[:], dstc, ADD)
                # tail
                if k < 3:
                    if HAS_BH:
                        for t in range(TPB):
                            nreal = min(npc - t * 128, 128)
                            if nreal > 0:
                                nc.vector.memset(
                                    ATacc[0:nreal, t * 384 + H: t * 384 + H + 1], 1.0)
                    for t in range(TPB):
                        trsb = transpose_tile(ATacc, t)
                        b_ps = ps_big.tile([128, 384], f32, tag="bps")
                        for m in range(3):
                            lhs = trsb[0:(128 if m < 2 else 64),
                                       m * 128:(m + 1) * 128]
                            nc.tensor.matmul(
                                b_ps[:], lhs, wht_s[m][:],
                                start=(m == 0), stop=(m == 2),
                                skip_group_check=True)
                        if k == 1:
                            bsb = mpool.tile([128, 384], bf16, tag="bsb")
                            nc.vector.tensor_copy(bsb[:], b_ps[:])
                            nc.sync.dma_start(B2[t * 128:(t + 1) * 128, :], bsb[:])
                            if t % TPQ == TPQ - 1:
                                q = t // TPQ
                                nc.gpsimd.collective_compute(
                                    "AllGather", bass.mybir.AluOpType.bypass,
                                    replica_groups=[list(range(C))],
                                    ins=[B2[q * RPQ:(q + 1) * RPQ, :].opt()],
                                    outs=[Bfull_q[q].opt()])
                        else:
                            nc.vector.tensor_copy(
                                B3_sb[:, t * HW:(t + 1) * HW], b_ps[:, 0:HW])
                else:
                    # readout
                    acc = smallpool.tile([128, 3], f32, tag="acc")
                    nc.vector.memset(acc[:], 0.0)
                    for t in range(TPB):
                        atr = smallpool.tile([128, 128], bf16, tag="atr")
                        nc.sync.dma_start(atr[:], atomT_read[:, t * 128:(t + 1) * 128])
                        trsb = transpose_tile(ATacc, t)
                        ar_ps = ps_big.tile([128, 384], f32, tag="bps")
                        for m in range(3):
                            dstp = ar_ps[:, m * 128:(m + 1) * 128]
                            nc.tensor.matmul(dstp, wo_ka_s[:, m * 128:(m + 1) * 128],
                                             atr[:], start=(m == 0), stop=False,
                                             skip_group_check=True)
                            nc.tensor.matmul(dstp, wo_k2_s[:, m * 128:(m + 1) * 128],
                                             trsb[:, 0:128], start=False, stop=False,
                                             skip_group_check=True)
                            nc.tensor.matmul(dstp, wo_k3_s[:, m * 128:(m + 1) * 128],
                                             trsb[:, 128:256], start=False, stop=False,
                                             skip_group_check=True)
                            nc.tensor.matmul(dstp, wo_k4_s[:, m * 128:(m + 1) * 128],
                                             trsb[0:64, 256:384], start=False,
                                             stop=(m == 2), skip_group_check=True)
                        arsb = mpool.tile([128, 384], f32, tag="ar")
                        nc.vector.tensor_scalar(arsb[:], ar_ps[:], 0.0, None, MAX)
                        red = smallpool.tile([128, 3], f32, tag="red")
                        for m in range(3):
                            nc.vector.reduce_sum(
                                red[:, m:m + 1], arsb[:, m * 128:(m + 1) * 128],
                                axis=bass.mybir.AxisListType.X)
                        nc.vector.tensor_tensor(acc[:], red[:], acc[:], ADD)
                    accd = dram.tile([128, 3], f32)
                    accr_d = dram.tile([128, 3], f32)
                    accsb = smallpool.tile([128, 3], f32, tag="accr")
                    nc.sync.dma_start(accd[:], acc[:])
                    nc.gpsimd.collective_compute(
                        "AllReduce", bass.mybir.AluOpType.add,
                        replica_groups=[list(range(C))],
                        ins=[accd.opt()], outs=[accr_d.opt()])
                    nc.sync.dma_start(accsb[:], accr_d[:])
                    o_ps = ps_big.tile([1, 320], f32, tag="bps")
                    for cc in range(3):
                        nc.tensor.matmul(o_ps[:], accsb[:, cc:cc + 1], woutt_s[cc][:],
                                         start=(cc == 0), stop=False,
                                         skip_group_check=True)
                    nc.tensor.matmul(o_ps[:], one_s[:], bout_s[:],
                                     start=False, stop=True, skip_group_check=True)
                    osb = smallpool.tile([1, 320], f32, tag="osb")
                    nc.vector.tensor_scalar(osb[:], o_ps[:], 0.0, None, MAX)
                    nc.sync.dma_start(out_d[:], osb[:])

            sweep1()
            sweep(2)
            sweep(3)

    nc.compile()
    return nc


_last_results = None


def kernel(**inputs):
    """Full-shape entry point: returns [300] float32."""
    global _last_results
    trace = bool(inputs.pop("_trace", False))
    atom = np.asarray(inputs["atom_features"], np.float32)
    ef = np.asarray(inputs["edge_features"], np.float32)
    src = np.asarray(inputs["edge_src"]).astype(np.int64)
    dst = np.asarray(inputs["edge_dst"]).astype(np.int64)
    args = [atom, ef, src, dst] + [np.asarray(inputs[k], np.float32) for k in
                                   ("Wi", "bi", "Wh", "bh", "Wo", "bo", "Wout", "bout")]
    meta, in_maps = preprocess(*args)
    nc = build_nc(meta)
    from concourse.bass_utils import run_bass_kernel_spmd
    res = run_bass_kernel_spmd(nc, in_maps, list(range(meta["C"])), trace=trace)
    _last_results = res
    out = np.asarray(res.results[0]["out"]).reshape(-1)[:H].astype(np.float32)
    return out
